# revision 1
# baseline (speedup 1.0000x reference)
"""Trainium2 Bass kernel for nn_NSMCell (GNN message passing).

Strategy
--------
The reference output is only [N]: a per-graph blend of two segment softmaxes
over per-node scalars.  Both scalars are of the form

    s_i = sum_d w_d * elu( M_g[d, :] @ x_i )

where for "node items" M_g = (sim[g] . W_node_props) * instr[g] and x = node
attr, and for "edge items" M_g = W_edge * instr[g] and x = edge attr.  The
per-graph matrices are built on the host (they are tiny); the device streams
all item columns through matmuls + exp/min elu + a weighted partition
reduce.  The edge-message scatter (index_add) collapses to a host-side
bincount of per-edge scalars, and the segment softmax + blend run on the
host over [N] values (negligible work).

Sharding: graphs are ranked by edge count and dealt round-robin so core d
gets slot-j graph rank 8j+d.  All 8 cores share one NEFF: per-slot run
lengths are the max over the 8 cores' graphs in that slot (tight padding,
~88 tiles of 512 items vs 96 for per-graph pow-2 padding).  Items are
packed [edge slots 8-15 | all node runs | edge slots 0-7] so only the
first 1 MB weight chunk gates startup; per-graph y matmuls address
variable column ranges inside each 512-item tile (weight reloads are free:
LDWEIGHTS overlaps any matmul with >= ~245 free columns, and run placement
keeps pieces >= 64 columns).

Device layout per 512-item tile (d on partitions, 2 chunks of 128 side by
side in one 2-bank PSUM tile):
  y[d, e]   = A_seg[k, d]^T @ xT[k, e]      2 matmuls per (piece, dc) -> PSUM
  E         = exp(y)                        ScalarE, PSUM -> SBUF bf16
  EL1       = min(E, relu(y)+1) = elu(y)+1  one fused custom VectorE op
  s-rows   += (w (x) delta_c)^T @ EL1       2 matmuls into a PSUM s-bank
The s-bank accumulates one 512-wide row per tile (col-group trick), drained
once at the end; the host subtracts sum(w) to undo the +1.

Item DMAs alternate between the SP and Pool HWDGE queues: one queue caps at
~175 GB/s, two sustain ~260 GB/s, keeping the PE stream fed.
"""

import numpy as np
import ml_dtypes

BF16 = ml_dtypes.bfloat16
N_CORES = 8
D = 256
TILE = 512  # items per tile


# ----------------------------------------------------------------------------
# Bass kernel builder (one NEFF shared by all cores)
# ----------------------------------------------------------------------------

_BASS_CACHE = {}


def _get_elup1_op():
    """Register (once) a custom fused DVE op: out = min(in0, relu(in1) + s0).

    With in0 = exp(y) and in1 = y this computes elu(y) + 1 in a single
    VectorE pass, replacing a tensor_scalar + tensor_tensor pair."""
    from concourse import dve_ops
    from concourse.dve_spec import (Spec, Src0, Src1, C0, relu, minn, lower,
                                    _has_src1)
    from concourse.dve_uop import DveOpSpec

    for o in dve_ops.OPS:
        if o.name == "ELUP1_ANT":
            return o

    def ref(in0, in1, s0, s1, imm2):
        return np.minimum(
            in0.astype(np.float32),
            np.maximum(in1.astype(np.float32), 0.0) + s0,
        ).astype(np.float32)

    spec = Spec(body=minn(Src0, relu(Src1) + C0), reference=ref)
    row = dve_ops._CUSTOM_DVE_ROW_BASE + len(dve_ops.OPS)
    shas = {}
    for ver in ("v3", "v4"):
        uops = lower(spec, ver=ver)
        shas[ver] = DveOpSpec(name="ELUP1_ANT", opcode=row, uops=uops,
                              rd1_en=_has_src1(spec)).sha(ver)
    op = dve_ops.DveOp("ELUP1_ANT", spec, subdim=False, uops_sha=shas)
    dve_ops.OPS.append(op)
    dve_ops.CUSTOM_DVE_SPECS[op.name] = op.spec
    dve_ops._SUB_OPCODE_FOR_NAME[op.name] = row
    return op


def _build_bass(n_tiles, pieces, stypes):
    """Build the Tile/Bass program.

    n_tiles: number of 512-item tiles per core
    pieces:  per tile, tuple of (a, b, u): y-matmul column range [a, b) using
             per-graph matrix u (u = slot for nodes, 16 + slot for edges)
    stypes:  per tile, tuple of (ra, rb, typ): s-reduce column ranges by
             item type (0 = node -> w_node, 1 = edge -> w_rel)
    """
    key = (n_tiles, pieces, stypes)
    if key in _BASS_CACHE:
        return _BASS_CACHE[key]

    import concourse.mybir as mybir
    import concourse.tile as tile
    from concourse import bacc

    dt = mybir.dt
    n_seg = 32  # 16 slots x (node, edge), u-ordered: nodes 0-15, edges 16-31
    assert n_tiles <= 128

    elup1 = _get_elup1_op()
    nc = bacc.Bacc("TRN2", target_bir_lowering=False)
    m_pad = n_tiles * TILE
    items_d = nc.dram_tensor("items", [128, 2 * m_pad], dt.bfloat16,
                             kind="ExternalInput")
    mats_d = nc.dram_tensor("mats", [128, n_seg * 2 * 2 * 128], dt.bfloat16,
                            kind="ExternalInput")
    wtab_d = nc.dram_tensor("wtab", [128, 2 * 2 * 32 * 32], dt.bfloat16,
                            kind="ExternalInput")
    s_d = nc.dram_tensor("s_out", [128, TILE], dt.float32,
                         kind="ExternalOutput")

    with tile.TileContext(nc) as tc:
        with (
            tc.tile_pool(name="const", bufs=1) as const_pool,
            tc.tile_pool(name="items", bufs=12) as item_pool,
            tc.tile_pool(name="psum_y", bufs=3, space="PSUM") as ypool,
            tc.tile_pool(name="psum_s", bufs=1, space="PSUM") as spool,
            tc.tile_pool(name="elu", bufs=8) as elu_pool,
            tc.tile_pool(name="sout", bufs=1) as sout_pool,
            tc.tile_pool(name="warm", bufs=1, space="PSUM") as warm_pool,
        ):
            # Consts: one pool tile per chunk so the chunk DMAs carry no
            # same-tile WAW deps (they'd serialize otherwise).  mats chunk 0
            # goes on the idle SP queue ahead of the item stream; the rest
            # stream on the ACT HWDGE path.
            MCH = 8  # u-slots per mats chunk
            mats_sbs = [const_pool.tile([128, MCH * 512], dt.bfloat16,
                                        name=f"matsb{i}", tag=f"mats{i}")
                        for i in range(n_seg // MCH)]
            wtab_sbs = [const_pool.tile([128, 8 * 128], dt.bfloat16,
                                        name=f"wtabb{i}", tag=f"wtab{i}")
                        for i in range(4)]

            def load_mats(ch, eng=None):
                sl = slice(ch * MCH * 512, (ch + 1) * MCH * 512)
                (eng or nc.scalar).dma_start(mats_sbs[ch][:], mats_d[:, sl])

            def load_mats_q(ch, q, eng):
                # one quarter of a mats chunk - slipped between item DMAs
                w = MCH * 512 // 4
                sl = slice(ch * MCH * 512 + q * w, ch * MCH * 512 + (q + 1) * w)
                eng.dma_start(mats_sbs[ch][:, q * w:(q + 1) * w],
                              mats_d[:, sl])

            def load_wtab(ch):
                sl = slice(ch * 8 * 128, (ch + 1) * 8 * 128)
                nc.scalar.dma_start(wtab_sbs[ch][:, :], wtab_d[:, sl])

            # HAM pre-warm: keep PE busy during the DMA preamble so real
            # matmuls start at 2.4 GHz instead of ramping from 1.2 GHz.
            warm_sb = const_pool.tile([128, 64], dt.bfloat16)
            nc.vector.memset(warm_sb[:], 0)
            warm_ps = warm_pool.tile([128, 64], dt.float32)
            for _ in range(56):
                nc.tensor.matmul(warm_ps[0:64, :], warm_sb[:], warm_sb[:],
                                 start=True, stop=True, skip_group_check=True)

            # Only mats chunk 0 gates the first region.  Its first quarters
            # lead the two item queues; the rest slots in between the first
            # item tiles.  wtab + chunk 1 dribble on the idle ACT path;
            # chunks 2/3 slip between item DMAs in quarters well before
            # their first-use tiles.
            first_use = {ch: n_tiles for ch in range(4)}
            for t in range(n_tiles):
                for (_, _, u) in pieces[t]:
                    first_use[u // MCH] = min(first_use[u // MCH], t)
            load_mats(0, nc.sync)
            load_wtab(0)
            load_wtab(1)
            load_wtab(2)
            load_wtab(3)
            load_mats(1)
            const_sched = {}
            # slip these in only after the third item queue joins (tile 18):
            # earlier insertions eat the item prefetch lead while it is small
            for ch in (2, 3):
                t0 = max(22, first_use[ch] - 20)
                for q, dt_ in enumerate((0, 1, 4, 5)):
                    tq = t0 + dt_
                    eng = nc.sync if tq % 2 == 0 else nc.gpsimd
                    const_sched.setdefault(tq, []).append(
                        (lambda ch=ch, q=q, eng=eng:
                         load_mats_q(ch, q, eng)))
            psum_s = spool.tile([128, TILE], dt.float32)

            def mat_sl(u, kc, dc):
                ch, s = divmod(u, MCH)
                off = ((s * 2 + kc) * 2 + dc) * 128
                return mats_sbs[ch][:, off:off + 128]

            def w_sl(typ, kc, c):
                # c-major so tile t only depends on wtab chunk c // 8
                ch, cc = divmod(c, 8)
                off = ((cc * 2 + typ) * 2 + kc) * 32
                return wtab_sbs[ch][:, off:off + 32]

            grp_started = set()
            pending_s = []
            for t in range(n_tiles):
                grp, c = divmod(t, 32)

                for fn in const_sched.get(t, ()):
                    fn()
                x2 = item_pool.tile([128, 2 * TILE], dt.bfloat16, tag="x")
                xoff = 0
                # two queues early (scalar still streams wtab/ch1); rotate
                # through three once the ACT path is free so the item lead
                # grows instead of riding just-in-time on saturated queues
                if t < 18:
                    eng = nc.sync if t % 2 == 0 else nc.gpsimd
                else:
                    eng = (nc.sync, nc.gpsimd, nc.scalar)[(t - 18) % 3]
                eng.dma_start(
                    x2[:], items_d[:, t * 2 * TILE:(t + 1) * 2 * TILE])

                # both d-chunks side by side in one 2-bank PSUM tile
                y = ypool.tile([128, 2 * TILE], dt.float32, tag="y")
                n_p = len(pieces[t])
                for pi, (a, b, u) in enumerate(pieces[t]):
                    for dc in range(2):
                        ysl = y[:, dc * TILE + a:dc * TILE + b]
                        nc.tensor.matmul(ysl, mat_sl(u, 0, dc),
                                         x2[:, xoff + a:xoff + b],
                                         start=(pi == 0), stop=False,
                                         skip_group_check=True)
                        nc.tensor.matmul(ysl, mat_sl(u, 1, dc),
                                         x2[:, xoff + TILE + a:xoff + TILE + b],
                                         start=False,
                                         stop=(pi == n_p - 1),
                                         skip_group_check=True)
                e_t = elu_pool.tile([128, 2 * TILE], dt.bfloat16, tag="e")
                nc.scalar.activation(e_t[:], y[:],
                                     mybir.ActivationFunctionType.Exp)
                el_t = elu_pool.tile([128, 2 * TILE], dt.bfloat16, tag="el")
                nc.vector._custom_dve(elup1, out=el_t[:], in0=e_t[:],
                                      in1=y[:], s0=1.0)

                # defer this tile's s-reduce matmuls by one tile so the
                # ACT->DVE chain has a full tile of slack before PE needs el_t
                def s_mms(c=c, grp=grp, el_t=el_t, t=t, srs=stypes[t]):
                    out_rows = psum_s[32 * grp:32 * grp + 32, :]
                    tp = (0, 32 * grp)
                    last_t = (t == n_tiles - 1 or c == 31)
                    for ri, (ra, rb, typ) in enumerate(srs):
                        for kc in range(2):
                            st = grp not in grp_started
                            grp_started.add(grp)
                            stop = (last_t and ri == len(srs) - 1 and kc == 1)
                            nc.tensor.matmul(
                                out_rows[:, ra:rb], w_sl(typ, kc, c),
                                el_t[:, kc * TILE + ra:kc * TILE + rb],
                                start=st, stop=stop,
                                tile_position=tp, skip_group_check=True)
                pending_s.append(s_mms)
                if len(pending_s) > 2:
                    pending_s.pop(0)()

            for fn in pending_s:
                fn()

            s_sb = sout_pool.tile([128, TILE], dt.float32)
            nc.vector.tensor_copy(out=s_sb[0:64, 0:1], in_=warm_ps[0:64, 0:1])
            nc.vector.tensor_copy(out=s_sb[:], in_=psum_s[:])
            nc.scalar.dma_start(s_d[:], s_sb[:])

    nc.compile()
    _BASS_CACHE[key] = nc
    return nc


# ----------------------------------------------------------------------------
# Host-side wrapper
# ----------------------------------------------------------------------------

def kernel(instruction_batch, distribution, node_prop_similarities,
           relation_similarity, node_attrs, edge_attrs,
           W_node_props, W_edge, w_node_score, w_rel_score,
           node_indices, edge_batch_indices, edge_indices):
    from concourse.bass_utils import run_bass_kernel_spmd

    ib = np.asarray(instruction_batch, dtype=np.float32)
    dist = np.asarray(distribution, dtype=np.float32)
    sim = np.asarray(node_prop_similarities, dtype=np.float32)
    rsim = np.asarray(relation_similarity, dtype=np.float32)
    na = np.asarray(node_attrs, dtype=np.float32)
    ea = np.asarray(edge_attrs, dtype=np.float32)
    Wp = np.asarray(W_node_props, dtype=np.float32)
    We = np.asarray(W_edge, dtype=np.float32)
    wn = np.asarray(w_node_score, dtype=np.float32)
    wr = np.asarray(w_rel_score, dtype=np.float32)
    ni = np.asarray(node_indices).astype(np.int64)
    ebi = np.asarray(edge_batch_indices).astype(np.int64)
    ei = np.asarray(edge_indices).astype(np.int64)
    src, dst = ei[0], ei[1]

    B = ib.shape[0]
    N = na.shape[0]
    G = B // N_CORES  # graphs (slots) per core

    cn = np.bincount(ni, minlength=B)
    ce = np.bincount(ebi, minlength=B)
    nstart = np.concatenate([[0], np.cumsum(cn)])
    eperm = np.argsort(ebi, kind="stable")
    estart = np.concatenate([[0], np.cumsum(ce)])

    # ---- layout plan: rank graphs by edge count, slot j = ranks [8j, 8j+8)
    order = np.argsort(-ce, kind="stable")
    slot_graphs = order.reshape(G, N_CORES)          # [slot, dev] -> graph
    Ln = (-(-cn[slot_graphs].max(axis=1) // 4)) * 4  # per-slot node run len
    Le = (-(-ce[slot_graphs].max(axis=1) // 4)) * 4

    # Region order [edges slots 8-15 | nodes 0-15 | edges 0-7]: the first
    # third references only mats chunk 0, so only 1 MB of weights is on the
    # startup critical path; the rest streams during the first regions.
    # u (weight index, chunk = u//8): edges 8-15 -> 0-7, nodes -> 8-23,
    # edges 0-7 -> 24-31.
    MINP = 4  # LDWEIGHTS pipelines under even tiny matmuls; no snap needed

    def place(lens, o0):
        offs, lens2 = [], []
        o = int(o0)
        for ln in lens:
            ln = int(ln)
            r = o % TILE
            if r and TILE - r < MINP:
                o += TILE - r
            end = o + ln
            tail = end % TILE
            if end // TILE > o // TILE and 0 < tail < MINP:
                ln += MINP - tail
            offs.append(o)
            lens2.append(ln)
            o += ln
        return offs, lens2, o

    eoff_hi, Le2_hi, r1 = place([Le[j] for j in range(G // 2, G)], 0)
    noff, Ln2, r2 = place(Ln, r1)
    eoff_lo, Le2_lo, total = place([Le[j] for j in range(G // 2)], r2)
    eoff = eoff_lo + eoff_hi                # [slot] -> column offset
    Le2 = Le2_lo + Le2_hi
    n_tiles = -(-total // TILE)
    m_pad = n_tiles * TILE
    assert n_tiles <= 128, "s accumulator bank overflow"

    def u_of(j, typ):                       # weight index in use order
        if typ == 0:
            return 8 + j
        return 24 + j if j < G // 2 else j - G // 2

    runs = [(noff[j], Ln2[j], u_of(j, 0)) for j in range(G)] + \
           [(eoff[j], Le2[j], u_of(j, 1)) for j in range(G)]
    pieces = [[] for _ in range(n_tiles)]
    for (st, ln, u) in runs:
        if ln == 0:
            continue
        for t in range(st // TILE, (st + ln - 1) // TILE + 1):
            a = max(st, TILE * t) - TILE * t
            b = min(st + ln, TILE * (t + 1)) - TILE * t
            pieces[t].append((a, b, u))
    for p in pieces:
        p.sort()
    stypes = []
    for t in range(n_tiles):
        sr = []
        for (lo, hi, typ) in ((0, r1, 1), (r1, r2, 0), (r2, m_pad, 1)):
            a = max(lo, TILE * t) - TILE * t
            b = min(hi, TILE * (t + 1)) - TILE * t
            if a < b:
                sr.append((a, b, typ))
        stypes.append(tuple(sr))
    pieces = tuple(tuple(p) for p in pieces)
    stypes = tuple(stypes)

    # ---- item columns, transposed + bf16, packed per plan ----
    na_bf = na.astype(BF16)
    ea_bf = ea[eperm].astype(BF16)
    itemsv = np.zeros((N_CORES, 128, n_tiles, 2, TILE), dtype=BF16)

    def put(dev, col0, block):
        n = block.shape[0]
        bT = block.T.reshape(2, 128, n)  # [kc, p, n]
        j = np.arange(col0, col0 + n)
        tt, jj = j // TILE, j % TILE
        itemsv[dev][:, tt, 0, jj] = bT[0]
        itemsv[dev][:, tt, 1, jj] = bT[1]

    for j in range(G):
        for d in range(N_CORES):
            g = int(slot_graphs[j, d])
            put(d, int(noff[j]), na_bf[nstart[g]:nstart[g + 1]])
            put(d, int(eoff[j]), ea_bf[estart[g]:estart[g + 1]])
    items = itemsv.reshape(N_CORES, 128, 2 * m_pad)

    # ---- per-graph matrices A[k, d] (instr folded in), bf16 ----
    C = np.einsum("gp,pde->gde", sim, Wp)
    A_node = (C * ib[:, :, None]).transpose(0, 2, 1)           # [g, k, d]
    A_edge = (We[None, :, :] * ib[:, :, None]).transpose(0, 2, 1)
    A_all = np.empty((N_CORES, 2 * G, D, D), np.float32)       # [dev, u, k, d]
    for j in range(G):
        for d in range(N_CORES):
            g = int(slot_graphs[j, d])
            A_all[d, u_of(j, 0)] = A_node[g]
            A_all[d, u_of(j, 1)] = A_edge[g]
    # blob[p, ((u*2+kc)*2+dc)*128 + m] = A_u[kc*128+p][dc*128+m]
    Ar = A_all.reshape(N_CORES, 2 * G, 2, 128, 2, 128)  # dev,u,kc,p,dc,m
    mats = np.ascontiguousarray(Ar.transpose(0, 3, 1, 2, 4, 5)
                                ).reshape(N_CORES, 128, -1).astype(BF16)

    # ---- w tables: wtab[k, ((c*2+typ)*2+kc)*32+m] = w_typ[kc*128+k]*(m==c)
    wt = np.stack([wn, wr]).astype(np.float32)                  # [2, 256]
    eye = np.eye(32, dtype=np.float32)
    wtab = np.einsum("tk,cm->kctm", wt.reshape(2, 2, 128).reshape(4, 128), eye)
    wtab = np.ascontiguousarray(wtab.reshape(128, 32, 2, 2, 32)
                                ).reshape(128, 4 * 32 * 32).astype(BF16)

    # ---- run on 8 cores ----
    nc = _build_bass(n_tiles, pieces, stypes)
    in_maps = [{"items": items[d], "mats": mats[d], "wtab": wtab}
               for d in range(N_CORES)]
    res = run_bass_kernel_spmd(nc, in_maps, core_ids=list(range(N_CORES)))
    s_rows = np.stack([r["s_out"] for r in res.results])        # [8, 128, 512]

    # ---- unshard + finish on host ----
    sum_wn = float(wt[0].astype(BF16).astype(np.float32).sum())
    sum_wr = float(wt[1].astype(BF16).astype(np.float32).sum())
    state_logits = np.empty(N, np.float32)
    s_e = np.empty(ei.shape[1], np.float32)
    flat = s_rows.reshape(N_CORES, -1)
    for j in range(G):
        for d in range(N_CORES):
            g = int(slot_graphs[j, d])
            o = int(noff[j])
            state_logits[nstart[g]:nstart[g + 1]] = \
                flat[d][o:o + cn[g]] - sum_wn
            o = int(eoff[j])
            s_e[estart[g]:estart[g + 1]] = flat[d][o:o + ce[g]] - sum_wr

    rel_logits = np.bincount(dst[eperm], weights=dist[src[eperm]] * s_e,
                             minlength=N).astype(np.float32)

    def seg_softmax(x):
        mx = np.maximum.reduceat(x, nstart[:-1])
        ex = np.exp(x - mx[ni])
        sm = np.add.reduceat(ex, nstart[:-1])
        return ex / sm[ni]

    r = rsim[ni]
    out = r * seg_softmax(rel_logits) + (1.0 - r) * seg_softmax(state_logits)
    return out.astype(np.float32)

